# revision 1
# baseline (speedup 1.0000x reference)
# Multi-head attention block (projections + softmax attention + output
# projection + residual + LayerNorm) for Trainium2, 8 NeuronCores.
#
# Sharding: data-parallel. 8 cores = 4 batches x 2 query-halves. Core c
# handles batch c//2, query rows (c%2)*1024 .. +1024. Each core receives
# the full K/V of its batch (keys span the whole sequence) plus all
# weights, and produces its 1024 rows of the final output. No
# cross-core communication.
#
# Self-contained: hardcodes all shapes from the problem spec.
#   B, S, D, H = 4, 2048, 1024, 16 ; head_dim = 64 ; eps = 1e-6

from contextlib import ExitStack

import numpy as np

import concourse.bass as bass
import concourse.mybir as mybir
import concourse.tile as tile
from concourse import bacc
from concourse.bass_utils import run_bass_kernel_spmd
from concourse.masks import make_identity

B, S, D, H = 4, 2048, 1024, 16
HD = D // H          # 64 head dim
EPS = 1e-6
NCORES = 8
SQ = (B * S) // NCORES   # 1024 query rows per core
SK = S                   # 2048 keys per core
P = 128

FP32 = mybir.dt.float32
BF16 = mybir.dt.bfloat16

ET = D // P     # 8  e (input-feature) tiles
DT = D // P     # 8  d (output-feature) tiles
IT = SQ // P    # 8  query row-tiles
JT = SK // P    # 16 key row-tiles
IC = SQ // 512  # 2  query 512-chunks
EC = D // 512   # 2  feature 512-chunks
JC = SK // 512  # 4  key 512-chunks


def _emit(tc: tile.TileContext, ctx: ExitStack):
    nc = tc.nc

    Q = nc.dram_tensor("Q", [SQ, D], FP32, kind="ExternalInput").ap()
    K = nc.dram_tensor("K", [SK, D], FP32, kind="ExternalInput").ap()
    V = nc.dram_tensor("V", [SK, D], FP32, kind="ExternalInput").ap()
    Wq = nc.dram_tensor("Wq", [D, D], FP32, kind="ExternalInput").ap()
    Wk = nc.dram_tensor("Wk", [D, D], FP32, kind="ExternalInput").ap()
    Wv = nc.dram_tensor("Wv", [D, D], FP32, kind="ExternalInput").ap()
    Wo = nc.dram_tensor("Wo", [D, D], FP32, kind="ExternalInput").ap()
    gamma = nc.dram_tensor("ln_gamma", [D], FP32, kind="ExternalInput").ap()
    beta = nc.dram_tensor("ln_beta", [D], FP32, kind="ExternalInput").ap()
    out = nc.dram_tensor("out", [SQ, D], FP32, kind="ExternalOutput").ap()

    persist = ctx.enter_context(tc.tile_pool(name="persist", bufs=1))
    stage = ctx.enter_context(tc.tile_pool(name="stage", bufs=5))
    # one shared PSUM tag for transposes + projections + O-proj: 2 banks
    psum_p = ctx.enter_context(tc.tile_pool(name="psum_p", bufs=2, space="PSUM"))

    ident = persist.tile([P, P], FP32, tag="ident", name="ident")
    make_identity(nc, ident[:])

    gamma_b = persist.tile([P, D], FP32, tag="gamma_b", name="gamma_b")
    nc.gpsimd.dma_start(out=gamma_b[:], in_=gamma[None, :].to_broadcast((P, D)))
    beta_b = persist.tile([P, D], FP32, tag="beta_b", name="beta_b")
    nc.gpsimd.dma_start(out=beta_b[:], in_=beta[None, :].to_broadcast((P, D)))
    eps_t = persist.tile([P, 1], FP32, tag="eps_t", name="eps_t")
    nc.vector.memset(eps_t[:], EPS)

    def pp():
        return psum_p.tile([P, 512], FP32, tag="pp", name="pp")

    def load_rows(dram, r0):
        t = stage.tile([P, D], FP32, tag="stage", name="stage")
        nc.sync.dma_start(out=t[:], in_=dram[r0 : r0 + P, :])
        return t

    def transpose_in(dram, nrt, dst):
        # dst[p, ct, r] = dram[r, ct*128 + p], cast to bf16; dst is one
        # [128, ET, nrt*128] tile. Four 128x128 PE transposes share one
        # psum tile, evacuated by a single strided DVE copy.
        for rt in range(nrt):
            st = load_rows(dram, rt * P)
            for eg in range(ET // 4):
                ps = pp()
                for k in range(4):
                    nc.tensor.transpose(
                        ps[:, k * P : (k + 1) * P],
                        st[:, (4 * eg + k) * P : (4 * eg + k + 1) * P],
                        ident[:],
                    )
                nc.vector.tensor_copy(
                    out=dst[:, 4 * eg : 4 * eg + 4, rt * P : (rt + 1) * P],
                    in_=ps[:].rearrange("p (k r) -> p k r", r=P),
                )

    # ---- persistent tensors ----
    qT = [persist.tile([P, SQ], BF16, tag=f"qt{i}", name=f"qt{i}") for i in range(DT)]
    v_sb = [persist.tile([P, H, HD + 1], BF16, tag=f"v{j}", name=f"v{j}") for j in range(JT)]
    outT = [persist.tile([P, SQ], BF16, tag=f"ot{i}", name=f"ot{i}") for i in range(DT)]

    # ---- K prelude: WkT + KT stay alive through the attention loop ----
    k_ctx = ExitStack()
    wkp = k_ctx.enter_context(tc.tile_pool(name="wk", bufs=1))
    ktr = k_ctx.enter_context(tc.tile_pool(name="ktrans", bufs=1))
    WkT = wkp.tile([P, ET, D], BF16, tag="wkt", name="wkt")
    transpose_in(Wk, DT, WkT)
    KT = ktr.tile([P, ET, SK], BF16, tag="KT", name="KT")
    transpose_in(K, JT, KT)

    # ---- V: transpose + project (natural [j, d] layout + ones column) ----
    for jt in range(JT):
        nc.gpsimd.memset(v_sb[jt][:], 1.0)
    with tc.tile_pool(name="wv", bufs=1) as wvp:
        WvT = wvp.tile([P, ET, D], BF16, tag="wvt", name="wvt")
        transpose_in(Wv, DT, WvT)
        with (
            tc.tile_pool(name="vtrans", bufs=1) as vtr,
            tc.tile_pool(name="vpsum", bufs=4, space="PSUM") as vps,
        ):
            VT = vtr.tile([P, ET, SK], BF16, tag="VT", name="VT")
            transpose_in(V, JT, VT)
            # v[j, d] = sum_e V[j, e] * Wv[d, e]; one ldweights per (et, jt)
            for jb in range(JT // 2):
                ps = [vps.tile([P, 512], FP32, tag="vp", name="vp") for _ in range(4)]
                for et in range(ET):
                    for u in range(2):
                        jt = 2 * jb + u
                        for dc in range(EC):
                            nc.tensor.matmul(
                                ps[2 * u + dc][:],
                                VT[:, et, jt * P : (jt + 1) * P],
                                WvT[:, et, dc * 512 : (dc + 1) * 512],
                                start=(et == 0),
                                stop=(et == ET - 1),
                            )
                for u in range(2):
                    jt = 2 * jb + u
                    for dc in range(EC):
                        nc.scalar.copy(
                            out=v_sb[jt][:, dc * 8 : (dc + 1) * 8, 0:HD],
                            in_=ps[2 * u + dc][:].rearrange("p (h d) -> p h d", d=HD),
                        )

    # ---- Q: transpose + project ----
    with tc.tile_pool(name="wq", bufs=1) as wqp:
        WqT = wqp.tile([P, ET, D], BF16, tag="wqt", name="wqt")
        transpose_in(Wq, DT, WqT)
        with (
            tc.tile_pool(name="qtrans", bufs=1) as qtr,
            tc.tile_pool(name="qpsum", bufs=2, space="PSUM") as qps,
        ):
            QT = qtr.tile([P, ET, SQ], BF16, tag="QT", name="QT")
            transpose_in(Q, IT, QT)
            # qT[dt][p_d, i] = sum_e Wq[d, e] * Q[i, e]
            for dt in range(DT):
                ps = [qps.tile([P, 512], FP32, tag="qp", name="qp") for _ in range(IC)]
                for et in range(ET):
                    for icc in range(IC):
                        nc.tensor.matmul(
                            ps[icc][:],
                            WqT[:, et, dt * P : (dt + 1) * P],
                            QT[:, et, icc * 512 : (icc + 1) * 512],
                            start=(et == 0),
                            stop=(et == ET - 1),
                        )
                for icc in range(IC):
                    nc.scalar.copy(
                        out=qT[dt][:, icc * 512 : (icc + 1) * 512], in_=ps[icc][:]
                    )

    # WoT[p, dt, e] = Wo[e, dt*128+p] -- emitted here so the load +
    # transpose overlap the attention phase instead of gating the tail
    WoT = persist.tile([P, ET, D], BF16, tag="wot", name="wot")
    transpose_in(Wo, DT, WoT)

    # ---- attention, head pair by head pair; k-proj interleaved as PE filler ----
    attn_ctx = ExitStack()
    ktp_pool = attn_ctx.enter_context(tc.tile_pool(name="ktp", bufs=4))
    expt_pool = attn_ctx.enter_context(tc.tile_pool(name="expt", bufs=6))
    norm_pool = attn_ctx.enter_context(tc.tile_pool(name="norm", bufs=3))
    psum_s = attn_ctx.enter_context(tc.tile_pool(name="psum_s", bufs=2, space="PSUM"))
    psum_o = attn_ctx.enter_context(tc.tile_pool(name="psum_o", bufs=2, space="PSUM"))
    dram_sc = attn_ctx.enter_context(tc.tile_pool(name="dram_sc", bufs=4, space="DRAM"))

    for dt in range(DT):  # head pair
        # k-proj for this pair, written directly into the zero-padded
        # per-head tiles: head 2dt on partitions 0:64 of ktp_a, head
        # 2dt+1 on partitions 64:128 of ktp_b, zeros elsewhere.
        ktp_a = ktp_pool.tile([P, SK], BF16, tag="ktp", name="ktp_a")
        ktp_b = ktp_pool.tile([P, SK], BF16, tag="ktp", name="ktp_b")
        nc.gpsimd.memset(ktp_a[:], 0.0)
        nc.gpsimd.memset(ktp_b[:], 0.0)
        for jch in range(2):
            ps = [pp() for _ in range(2)]
            for et in range(ET):
                for u in range(2):
                    nc.tensor.matmul(
                        ps[u][:],
                        WkT[:, et, dt * P : (dt + 1) * P],
                        KT[:, et, (2 * jch + u) * 512 : (2 * jch + u + 1) * 512],
                        start=(et == 0),
                        stop=(et == ET - 1),
                    )
            for u in range(2):
                jsl = slice((2 * jch + u) * 512, (2 * jch + u + 1) * 512)
                nc.vector.tensor_copy(out=ktp_a[0:HD, jsl], in_=ps[u][0:HD, :])
                nc.vector.tensor_copy(out=ktp_b[HD:P, jsl], in_=ps[u][HD:P, :])

        for hh in range(2):
            h = 2 * dt + hh
            ktp = ktp_a if hh == 0 else ktp_b
            po = [psum_o.tile([P, 512], FP32, tag="po", name="po") for _ in range(IC)]
            for jt in range(JT):
                pscore = psum_s.tile([P, 1024], FP32, tag="ps", name="ps")
                for icc in range(IC):
                    # scoresT[j, i] = sum_d k_h[j, d] q_h[i, d]
                    nc.tensor.matmul(
                        pscore[:, icc * 512 : (icc + 1) * 512],
                        ktp[:, jt * P : (jt + 1) * P],
                        qT[dt][:, icc * 512 : (icc + 1) * 512],
                        start=True,
                        stop=True,
                    )
                expt = expt_pool.tile([P, 1024], BF16, tag="expt", name="expt")
                nc.scalar.activation(
                    out=expt[:],
                    in_=pscore[:],
                    func=mybir.ActivationFunctionType.Exp,
                    scale=0.125,  # 1/sqrt(64)
                )
                for icc in range(IC):
                    # o_unnorm[d, i] (+ row 64 = softmax denom l[i])
                    nc.tensor.matmul(
                        po[icc][0 : HD + 1, :],
                        v_sb[jt][:, h, :],
                        expt[:, icc * 512 : (icc + 1) * 512],
                        start=(jt == 0),
                        stop=(jt == JT - 1),
                    )
            # evacuate the attnV psum immediately (one copy) so the psum
            # banks free up for the next head; normalize off the copy
            pox = []
            for icc in range(IC):
                px = norm_pool.tile([P, 512], FP32, tag="pox", name="pox")
                nc.vector.tensor_copy(out=px[0 : HD + 1, :], in_=po[icc][0 : HD + 1, :])
                pox.append(px)
            # normalize by the softmax denominator; fill outT rows
            for icc in range(IC):
                isl = slice(icc * 512, (icc + 1) * 512)
                # partition-broadcast the denominator row via a DRAM
                # bounce (SBUF sources can't use a zero partition step),
                # then take the reciprocal on 64 lanes
                rl_d = dram_sc.tile([1, 512], FP32, tag="rl_d", name="rl_d")
                nc.sync.dma_start(out=rl_d[:], in_=pox[icc][HD : HD + 1, :])
                rlb = norm_pool.tile([P, 512], FP32, tag="rlb", name="rlb")
                nc.gpsimd.dma_start(
                    out=rlb[0:HD, :], in_=rl_d[:].to_broadcast((HD, 512))
                )
                # ~18-bit reciprocal: plenty for softmax denominators
                # (values are positive sums in [~1, ~1e5])
                nc.vector.reciprocal_approx_fast(out=rlb[0:HD, :], in_=rlb[0:HD, :])
                if hh == 0:
                    nc.vector.tensor_mul(
                        out=outT[dt][0:HD, isl],
                        in0=pox[icc][0:HD, :],
                        in1=rlb[0:HD, :],
                    )
                else:
                    # matmul output lives on partitions 0..64; shift to
                    # the upper half of the outT tile via DMA
                    tmp = norm_pool.tile([P, 512], BF16, tag="tmp", name="tmp")
                    nc.vector.tensor_mul(
                        out=tmp[0:HD, :], in0=pox[icc][0:HD, :], in1=rlb[0:HD, :]
                    )
                    nc.sync.dma_start(out=outT[dt][HD:P, isl], in_=tmp[0:HD, :])

    attn_ctx.close()
    k_ctx.close()

    # ---- output projection + residual + LayerNorm ----
    ln_pool = ctx.enter_context(tc.tile_pool(name="ln", bufs=3))

    for it in range(IT):
        rq = stage.tile([P, D], FP32, tag="stage", name="stage")
        nc.sync.dma_start(out=rq[:], in_=Q[it * P : (it + 1) * P, :])
        f = ln_pool.tile([P, D], FP32, tag="f", name="f")
        ps = [pp() for _ in range(EC)]
        for dt in range(DT):
            for ecc in range(EC):
                nc.tensor.matmul(
                    ps[ecc][:],
                    outT[dt][:, it * P : (it + 1) * P],
                    WoT[:, dt, ecc * 512 : (ecc + 1) * 512],
                    start=(dt == 0),
                    stop=(dt == DT - 1),
                )
        for ecc in range(EC):
            nc.vector.tensor_add(
                out=f[:, ecc * 512 : (ecc + 1) * 512],
                in0=ps[ecc][:],
                in1=rq[:, ecc * 512 : (ecc + 1) * 512],
            )
        stats = ln_pool.tile([P, 2, 6], FP32, tag="stats", name="stats")
        fv = f[:].rearrange("p (s x) -> p s x", s=2)
        for s_ in range(2):
            nc.vector.bn_stats(out=stats[:, s_, :], in_=fv[:, s_, :])
        mv = ln_pool.tile([P, 2], FP32, tag="mv", name="mv")
        nc.vector.bn_aggr(out=mv[:], in_=stats[:])
        rstd = ln_pool.tile([P, 1], FP32, tag="rstd", name="rstd")
        nc.scalar.activation(
            out=rstd[:],
            in_=mv[:, 1:2],
            func=mybir.ActivationFunctionType.Sqrt,
            bias=eps_t[:],
            scale=1.0,
        )
        nc.vector.reciprocal(out=rstd[:], in_=rstd[:])
        o_sb = ln_pool.tile([P, D], FP32, tag="o", name="o")
        nc.vector.tensor_scalar(
            out=o_sb[:],
            in0=f[:],
            scalar1=mv[:, 0:1],
            scalar2=rstd[:],
            op0=mybir.AluOpType.subtract,
            op1=mybir.AluOpType.mult,
        )
        nc.gpsimd.tensor_mul(out=o_sb[:], in0=o_sb[:], in1=gamma_b[:])
        nc.gpsimd.tensor_add(out=o_sb[:], in0=o_sb[:], in1=beta_b[:])
        nc.sync.dma_start(out=out[it * P : (it + 1) * P, :], in_=o_sb[:])


_CACHE = {}


def build_program():
    if "nc" not in _CACHE:
        nc = bacc.Bacc(
            "TRN2",
            target_bir_lowering=False,
            debug=False,
            enable_asserts=False,
            num_devices=NCORES,
        )
        with tile.TileContext(nc) as tc, ExitStack() as ctx:
            _emit(tc, ctx)
        nc.compile()
        _CACHE["nc"] = nc
    return _CACHE["nc"]


def shard_inputs(inputs):
    arr = {k: np.ascontiguousarray(np.asarray(v, dtype=np.float32)) for k, v in inputs.items()}
    in_maps = []
    for c in range(NCORES):
        b, hf = c // 2, c % 2
        in_maps.append(
            {
                "Q": np.ascontiguousarray(arr["Q"][b, hf * SQ : (hf + 1) * SQ, :]),
                "K": arr["K"][b],
                "V": arr["V"][b],
                "Wq": arr["Wq"],
                "Wk": arr["Wk"],
                "Wv": arr["Wv"],
                "Wo": arr["Wo"],
                "ln_gamma": arr["ln_gamma"],
                "ln_beta": arr["ln_beta"],
            }
        )
    return in_maps


def unshard_outputs(results):
    full = np.zeros((B, S, D), np.float32)
    for c in range(NCORES):
        b, hf = c // 2, c % 2
        full[b, hf * SQ : (hf + 1) * SQ, :] = results[c]["out"]
    return full


def kernel(**inputs):
    nc = build_program()
    in_maps = shard_inputs(inputs)
    res = run_bass_kernel_spmd(nc, in_maps, list(range(NCORES)))
    return unshard_outputs(res.results)


if __name__ == "__main__":
    rng = np.random.default_rng(0)
    ins = {
        "Q": rng.standard_normal((B, S, D), np.float32),
        "K": rng.standard_normal((B, S, D), np.float32),
        "V": rng.standard_normal((B, S, D), np.float32),
        "Wq": rng.standard_normal((D, D), np.float32) / np.sqrt(D),
        "Wk": rng.standard_normal((D, D), np.float32) / np.sqrt(D),
        "Wv": rng.standard_normal((D, D), np.float32) / np.sqrt(D),
        "Wo": rng.standard_normal((D, D), np.float32) / np.sqrt(D),
        "ln_gamma": np.ones(D, np.float32),
        "ln_beta": np.zeros(D, np.float32),
    }
    out = kernel(**ins)
    print(out.shape, out.dtype, np.abs(out).max())

